# revision 23
# baseline (speedup 1.0000x reference)
"""Trainium2 Bass kernel for nn_AllLoss_13400297964003.

Strategy (exact algebraic refactor of the reference loss):
  - The mask BCE term per anchor m is
        mean_{512x512}( softplus(up) - goal*up )
    with up = 4x nearest-upsample of z_m = coef_m . proto.  This equals
        ( 16*sum_ij softplus(z_m[ij]) - sum_ij z_m[ij]*G_m[ij] ) / 512^2
    where G_m = 4x4 block-sum pooling of gt_masks[gt_idx[m]].
  - The goal term collapses:  sum_m sum_ij z_m*G_m = sum_{p,g} C[p,g]*D[p,g]
    with C[p,g] = sum_{m: gt_idx[m]=g} coef[m,p]  (tiny, host-aggregated)
    and  D[p,g] = sum_ij proto[p,ij] * pool4x4(mask_g)[ij]  (device).
  - Sharding over 8 cores: core c gets anchors [32c,32c+32), gt masks
    [4c,4c+4), and 96 negative anchors; host combines scalars in float64.

v3 device plan:
  - gt_masks values are exactly 0.0/1.0, so the host shards them as bf16
    (a lossless re-encoding for these values).  Each core streams 2.1MB
    instead of 4.2MB, and the 4x4 row-pool matmuls run in bf16 (PSUM
    accumulation in f32: sums <=16 of 0/1 are exact).
  - All DMA on hardware-DGE queues: zin rides FIRST on the sync queue,
    ahead of the 4 whole-mask DMAs [128,2048]bf16 — the ring is FIFO, so
    zin is guaranteed to land before the mask flood saturates the SDMA
    engines (in the previous revision zin's tail landed ~3us late behind
    mask packets, sliding the entire z->Exp->Ln chain right).
    small8/proto_cat ride the scalar queue in parallel; result on sync.
    8 DMAs total (v1 had 14, all mask traffic on the gpsimd software-DGE
    queue whose Q7 descriptor emission serialized ~6us).
  - ACT is the critical engine: softplus = Exp then Ln(bias=1) with
    accum_out, (N+352)/1.2ns per op, dtype-independent — ~12us busy
    including two table loads.  All Exp-set ops are chain-ordered before
    all Ln-set ops so the spline tables load exactly twice (without the
    chain the scheduler interleaves the small cls Exp/Ln ops among the
    big z ops and the tables thrash: 4 loads, +2.6us).
  - The D-partial multiply/reduce for masks 0-2 runs on the otherwise
    idle gpsimd engine so the last mask's chain (which gates the result
    DMA) is never queued behind them on DVE.
  - Pool matmuls are dep-ordered after the z matmuls so the PE runs the
    z tiles first and the Exp chain starts as early as possible (without
    this the Tile scheduler interleaves pools first and the whole ACT
    chain slides into the tail).
  - log10 for the loc targets is folded into the host-side index gather
    (ln of gathered box sizes); the device computes the full smooth-L1.
"""

import numpy as np

N_CORES = 8
M = 256
NUM_GT = 32
M_LOC = M // N_CORES          # 32 anchors per core
G_LOC = NUM_GT // N_CORES     # 4 gt masks per core
NEG_LOC = 3 * M // N_CORES    # 96 negative anchors per core
LN10 = float(np.log(10.0))
NCOL = 20                     # 2 soft, 1 cls, 1 loc, 16 ddot

_CACHE = {}


def _build_nc():
    from contextlib import ExitStack
    import concourse.tile as tile
    from concourse import bacc, mybir
    from concourse.tile import add_dep_helper

    f32 = mybir.dt.float32
    bf16 = mybir.dt.bfloat16
    AF = mybir.ActivationFunctionType
    ALU = mybir.AluOpType
    AX = mybir.AxisListType

    nc = bacc.Bacc("TRN2", target_bir_lowering=False, debug=False)

    masks = nc.dram_tensor("masks", [G_LOC, 512, 512], bf16, kind="ExternalInput").ap()
    zin_a = nc.dram_tensor("zin_a", [16, 1152], bf16, kind="ExternalInput").ap()
    zin_b = nc.dram_tensor("zin_b", [16, 3072], bf16, kind="ExternalInput").ap()
    proto_cat = nc.dram_tensor("proto_cat", [128, 512], bf16, kind="ExternalInput").ap()
    small8 = nc.dram_tensor("small8", [128, 8], f32, kind="ExternalInput").ap()
    res = nc.dram_tensor("res", [128, NCOL], f32, kind="ExternalOutput").ap()

    with tile.TileContext(nc) as tc:
        with ExitStack() as ctx:
            constp = ctx.enter_context(tc.tile_pool(name="constp", bufs=1))
            maskp = ctx.enter_context(tc.tile_pool(name="maskp", bufs=4))
            zps = ctx.enter_context(tc.tile_pool(name="zps", bufs=3, space="PSUM"))
            rps = ctx.enter_context(tc.tile_pool(name="rps", bufs=2, space="PSUM"))
            workp = ctx.enter_context(tc.tile_pool(name="workp", bufs=3))
            dumpp = ctx.enter_context(tc.tile_pool(name="dumpp", bufs=1))
            outp = ctx.enter_context(tc.tile_pool(name="outp", bufs=1))

            # ---- 0/1 row-pool matrix on gpsimd (no input deps, runs at t0) ----
            # sr[I, 128c+k] = 1 iff k == 32c + I//4, i.e. (4k-128c-I) in [-3,0].
            sr_t = constp.tile([128, 512], bf16)
            srt1 = workp.tile([128, 4, 128], bf16, tag="srg")
            ones_col = nc.const_aps.scalar_like(1.0, srt1[:, 0, 0:1])
            ones_b = ones_col.broadcast_to([128, 4, 128])
            nc.gpsimd.affine_select(
                srt1[:], ones_b, pattern=[[128, 4], [-4, 128]],
                compare_op=ALU.is_ge, fill=0.0, base=0, channel_multiplier=1)
            sr3 = sr_t[:].rearrange("p (c k) -> p c k", c=4)
            nc.gpsimd.affine_select(
                sr3, srt1[:], pattern=[[-128, 4], [4, 128]],
                compare_op=ALU.is_ge, fill=0.0, base=3, channel_multiplier=-1)

            # ---- zin first on the sync HWDGE queue: the ring is FIFO, so
            # it is guaranteed to complete before the mask flood.  Split in
            # two so the first z matmul pair's data (w16 + first quarter of
            # proto16) completes ~1us earlier than the full 132KB would ----
            zina_t = constp.tile([16, 1152], bf16)
            nc.sync.dma_start(zina_t[:], zin_a[:])
            zinb_t = constp.tile([16, 3072], bf16)
            nc.sync.dma_start(zinb_t[:], zin_b[:])
            w16_t = zina_t[:, 0:128]

            def proto16_cols(lo, hi):
                """proto16[:, lo:hi]: cols 0:1024 live in zin_a (offset 128),
                the rest in zin_b."""
                if hi <= 1024:
                    return zina_t[:, 128 + lo:128 + hi]
                return zinb_t[:, lo - 1024:hi - 1024]

            # ---- other small inputs in parallel on the scalar queue ----
            small8_t = constp.tile([128, 8], f32)
            nc.scalar.dma_start(small8_t[:], small8[:])
            proto_cat_t = constp.tile([128, 512], bf16)
            nc.scalar.dma_start(proto_cat_t[:], proto_cat[:])
            clsx_t = small8_t[:, 0:1]
            clssgn_t = small8_t[:, 1:2]
            locp_t = small8_t[:, 2:3]
            locu_t = small8_t[:, 3:4]
            locv_t = small8_t[:, 4:5]
            locw_t = small8_t[:, 5:6]

            # ---- whole-mask DMAs on the sync HWDGE queue ----
            chunk = {}
            for g in range(G_LOC):
                t = maskp.tile([128, 2048], bf16, tag="mask")
                src = masks[g, :, :].rearrange("(h i) J -> i h J", h=4)
                nc.sync.dma_start(t[:], src)
                for c in range(4):
                    chunk[(g, c)] = t[:, 512 * c:512 * (c + 1)]

            PS = outp.tile([128, NCOL], f32)

            exp_phase = []   # ACT ops using the Exp table set
            ln_phase = []    # ACT ops using the Ln table set

            # ---- classification column: softplus(+-logit) ----
            et = workp.tile([128, 1], f32, tag="sm1")
            exp_phase.append(
                nc.scalar.activation(et[:], clsx_t, AF.Exp, scale=clssgn_t))

            # ---- localization smooth-L1 column ----
            rw = workp.tile([128, 1], f32, tag="sm4")
            nc.vector.reciprocal(rw[:], locw_t)
            df = workp.tile([128, 1], f32, tag="sm5")
            nc.vector.tensor_sub(df[:], locu_t, locv_t)
            tgt = workp.tile([128, 1], f32, tag="sm6")
            nc.vector.tensor_mul(tgt[:], df[:], rw[:])
            d = workp.tile([128, 1], f32, tag="sm7")
            nc.vector.tensor_sub(d[:], locp_t, tgt[:])
            a_t = workp.tile([128, 1], f32, tag="sm8")
            exp_phase.append(nc.scalar.activation(a_t[:], d[:], AF.Abs))
            mn = workp.tile([128, 1], f32, tag="sm9")
            nc.vector.tensor_scalar(mn[:], a_t[:], 1.0, None, op0=ALU.min)
            amn = workp.tile([128, 1], f32, tag="sm10")
            nc.vector.tensor_sub(amn[:], a_t[:], mn[:])
            sq = workp.tile([128, 1], f32, tag="sm11")
            nc.vector.tensor_mul(sq[:], mn[:], mn[:])
            nc.vector.scalar_tensor_tensor(PS[:, 3:4], sq[:], 0.5, amn[:],
                                           op0=ALU.mult, op1=ALU.add)

            # ---- z matmuls (bf16) + softplus (Exp then Ln, accum_out) ----
            exm = dumpp.tile([128, 4096], f32, tag="ex")
            z_mms = []
            for b in range(4):
                zt = zps.tile([128, 1024], f32, tag="z")
                z_mms.append(nc.tensor.matmul(
                    zt[:, 0:512], w16_t,
                    proto16_cols(1024 * b, 1024 * b + 512),
                    start=True, stop=True))
                z_mms.append(nc.tensor.matmul(
                    zt[:, 512:1024], w16_t,
                    proto16_cols(1024 * b + 512, 1024 * (b + 1)),
                    start=True, stop=True))
                exp_phase.append(nc.scalar.activation(
                    exm[:, 1024 * b:1024 * (b + 1)], zt[:], AF.Exp))

            ln_phase.append(
                nc.scalar.activation(PS[:, 2:3], et[:], AF.Ln, bias=1.0))
            for b in range(2):
                ln_phase.append(nc.scalar.activation(
                    exm[:, 2048 * b:2048 * (b + 1)],
                    exm[:, 2048 * b:2048 * (b + 1)], AF.Ln,
                    bias=1.0, accum_out=PS[:, b:b + 1]))

            # chain the ACT program order: all Exp-set ops, then all Ln-set ops
            order = exp_phase + ln_phase
            for a, b2 in zip(order, order[1:]):
                add_dep_helper(b2.ins, a.ins, sync=False, reason="act-table-phase")

            # ---- mask pooling + D partials ----
            pc3 = proto_cat_t[:].rearrange("p (a k) -> p a k", a=4)
            for g in range(G_LOC):
                R = rps.tile([128, 512], f32, tag="r")
                for c in range(4):
                    mm = nc.tensor.matmul(
                        R[:],
                        sr_t[:, 128 * c:128 * (c + 1)],
                        chunk[(g, c)],
                        start=(c == 0), stop=(c == 3),
                    )
                    add_dep_helper(mm.ins, z_mms[-1].ins, sync=False,
                                   reason="z-first")
                r4 = R[:].rearrange("p (j four) -> p j four", four=4)
                Pg = workp.tile([128, 128], bf16, tag="Pg")
                with nc.allow_low_precision(
                        reason="pooled 0/1 mask sums <=16 are exact in bf16"):
                    nc.vector.tensor_reduce(Pg[:], r4, axis=AX.X, op=ALU.add)
                # the prod for masks 0-2 runs on the idle gpsimd engine
                # (gpsimd tensor_reduce only supports cross-partition
                # axes, so the reduce stays on DVE).
                eng = nc.vector if g == G_LOC - 1 else nc.gpsimd
                prod = workp.tile([128, 4, 128], bf16, tag="prod")
                pgb = Pg[:].unsqueeze(1).broadcast_to([128, 4, 128])
                eng.tensor_mul(prod[:], pgb, pc3)
                nc.vector.tensor_reduce(PS[:, 4 + 4 * g:8 + 4 * g], prod[:],
                                        axis=AX.X, op=ALU.add)

            # ---- write result ----
            nc.sync.dma_start(res[:], PS[:])

    nc.compile()
    return nc


def _get_nc():
    if "nc" not in _CACHE:
        _CACHE["nc"] = _build_nc()
    return _CACHE["nc"]


def _host_prep(inputs):
    """Pure index-driven gathers/packing. Returns per-core input maps plus
    the float64 C aggregation matrix used in the final scalar combine."""
    import ml_dtypes
    bf16 = ml_dtypes.bfloat16
    f32 = np.float32
    proto = np.asarray(inputs["proto_types"], f32)[0]        # (4,128,128)
    map_class = np.asarray(inputs["map_class"], f32)[0]      # (3,64,64)
    map_box = np.asarray(inputs["map_box"], f32)[0]          # (12,64,64)
    map_coef = np.asarray(inputs["map_coef"], f32)[0]        # (12,64,64)
    anchor_center = np.asarray(inputs["anchor_center"], f32)  # (2,64,64)
    anchor_box = np.asarray(inputs["anchor_box"], f32)       # (3,2)
    gt_boxes = np.asarray(inputs["gt_boxes"], f32)[0]        # (32,4)
    gt_masks = np.asarray(inputs["gt_masks"], f32)[0]        # (32,512,512)
    pos_idx = np.asarray(inputs["pos_idx"])
    gt_idx = np.asarray(inputs["gt_idx"])
    neg_idx = np.asarray(inputs["neg_idx"])

    r, hh, ww = pos_idx[:, 0], pos_idx[:, 1], pos_idx[:, 2]
    ch4 = r[:, None] * 4 + np.arange(4, dtype=r.dtype)[None, :]
    coef = map_coef[ch4, hh[:, None], ww[:, None]]           # (256,4)
    pred = map_box[ch4, hh[:, None], ww[:, None]]            # (256,4)
    logit_pos = map_class[r, hh, ww]                         # (256,)
    logit_neg = map_class[neg_idx[:, 0], neg_idx[:, 1], neg_idx[:, 2]]  # (768,)
    a_ch = anchor_center[0, hh, ww]
    a_cw = anchor_center[1, hh, ww]
    a_h = anchor_box[r, 0]
    a_w = anchor_box[r, 1]
    gt = gt_boxes[gt_idx]                                    # (256,4)

    # replicated tensors
    proto_flat = proto.reshape(4, 16384)
    proto16 = np.ascontiguousarray(
        proto_flat.reshape(4, 4, 4096).transpose(1, 0, 2).reshape(16, 4096)
    ).astype(bf16)
    proto_cat = np.ascontiguousarray(proto.transpose(1, 0, 2).reshape(128, 512)).astype(bf16)
    # gt_masks values are exactly 0.0/1.0: bf16 re-encoding is lossless.
    gt_masks_bf = gt_masks.astype(bf16)
    # C[p,g] aggregation (float64, host)
    C = np.zeros((4, NUM_GT), np.float64)
    for p in range(4):
        np.add.at(C[p], gt_idx, coef[:, p].astype(np.float64))

    in_maps = []
    for cidx in range(N_CORES):
        msel = slice(M_LOC * cidx, M_LOC * (cidx + 1))
        nsel = slice(NEG_LOC * cidx, NEG_LOC * (cidx + 1))
        coef_c = coef[msel]                                  # (32,4)
        w16 = np.zeros((16, 128), f32)
        for q in range(4):
            w16[4 * q:4 * q + 4, 32 * q:32 * q + 32] = coef_c.T
        zin_a = np.ascontiguousarray(
            np.concatenate([w16.astype(bf16), proto16[:, 0:1024]], axis=1))
        zin_b = np.ascontiguousarray(proto16[:, 1024:4096])
        small = np.zeros((128, 8), f32)
        small[:, 6] = 1.0
        small[:, 0] = np.concatenate([logit_pos[msel], logit_neg[nsel]])
        small[:, 1] = np.concatenate(
            [np.full(M_LOC, -1.0, f32), np.full(NEG_LOC, 1.0, f32)])
        # k-blocked loc packing: rows k*32 + j; the log10 for the box
        # size targets is folded into the gather (ln of gt/anchor sizes,
        # divided on-device by ln10 packed in locw rows 64:128).
        small[:, 2] = pred[msel].T.reshape(128)
        small[:, 3] = np.concatenate(
            [gt[msel, 0], gt[msel, 1],
             np.log(gt[msel, 2]), np.log(gt[msel, 3])])
        small[:, 4] = np.concatenate(
            [a_ch[msel], a_cw[msel],
             np.log(a_h[msel]), np.log(a_w[msel])])
        small[:, 5] = np.concatenate(
            [a_h[msel], a_w[msel],
             np.full(M_LOC, LN10, f32), np.full(M_LOC, LN10, f32)])
        in_maps.append({
            "masks": np.ascontiguousarray(gt_masks_bf[G_LOC * cidx:G_LOC * (cidx + 1)]),
            "zin_a": zin_a,
            "zin_b": zin_b,
            "proto_cat": proto_cat,
            "small8": small,
        })
    return in_maps, C


def _combine(results, C):
    """results: list of per-core {'res': [128, NCOL]} dicts. float64 combine."""
    s_soft = 0.0
    s_cls = 0.0
    s_loc = 0.0
    s_dot = 0.0
    for cidx in range(N_CORES):
        rc = np.asarray(results[cidx]["res"], np.float64)
        s_soft += rc[:, 0:2].sum()
        s_cls += rc[:, 2].sum()
        s_loc += rc[:, 3].sum()
        for g in range(G_LOC):
            for p in range(4):
                s_dot += C[p, G_LOC * cidx + g] * rc[:, 4 + 4 * g + p].sum()
    total = s_cls + s_loc + (16.0 * s_soft - s_dot) / 262144.0 / float(M)
    return np.array(total, dtype=np.float32)


def kernel(**inputs):
    from concourse.bass_utils import run_bass_kernel_spmd
    nc = _get_nc()
    in_maps, C = _host_prep(inputs)
    out = run_bass_kernel_spmd(nc, in_maps, list(range(N_CORES)))
    return _combine(out.results, C)
